# revision 52
# baseline (speedup 1.0000x reference)
"""EulerAttentionHead Trainium2 kernel (8 NeuronCores, SPMD).

Reference computation (B=4, S=4096, D=1024, H=128):
    Q = x @ Wq.T + bq ; K = x @ Wk.T + bk ; V = x @ Wv.T + bv
    theta_{q,k} = {Q,K} / (wavelengths + 1e-8) + phase_bias
    sim = cos(tq) @ cos(tk).T + sin(tq) @ sin(tk).T
    out = softmax(sim / sqrt(H)) @ V @ Wo.T + bo

Sharding: 8 cores = 4 batches x 2 query-halves. Each core handles one
batch's full key/value set (4096 keys) and 2048 queries. The host rolls
x so each core's query rows are rows 0:2048 of its input (softmax over
keys is permutation-invariant, so key order doesn't matter).

Host prep: x and the four weight matrices are cast to fp16 and
pre-transposed in numpy, so every device-side matmul operand already has
its contraction dim on SBUF partitions and all device DMAs are plain
contiguous loads (no xbar transposes).

Per-core dataflow (PE datapath fp16, fp32 PSUM accumulation):
  phase A: per 512-row chunk, Q.T/K.T/V.T = W.T-stationary matmuls over
    x.T; theta built with per-partition scale/bias; round(theta/2pi) via
    the fp32 magic-number trick; Cody-Waite cascade + add_range_wrap
    (custom DVE ops) reduce into the ACT Sin LUT domain [-pi, pi];
    cos(t) = sin(wrap(t + pi/2)). V is re-transposed to natural [k, h]
    layout on the PE with a ones column appended.
  phase B: per 512-query chunk, S.T tiles [k,128 x q,512] = Fk-stationary
    matmuls (two k-tiles paired per 2-bank PSUM tile so one ACT pass
    computes exp(S/sqrt(H) - 1) -> E.T fp16; the -1 keeps exp under fp16
    max and cancels in the softmax normalization). AV: lhsT = E.T (FWL),
    rhs = [V | ones], so the softmax denominator accumulates as PSUM
    column 128 for free. Raw [O | denom] is evicted to SBUF; the
    reciprocal runs on the otherwise-idle DVE during B.
  phase C: normalize O rows by recip (ACT scale during eviction),
    PE-transpose O, project with Wo.T after seeding PSUM with bo via an
    identity matmul, evict on alternating DVE/ACT, store on alternating
    HWDGE queues.

PSUM is rebalanced per phase with stack-scoped tile pools (A: proj+V
transpose, B: 3 double-bank S.T tiles + packed O accumulators, C: O
transpose + output tiles). Note start=True zeroes the whole 2KB PSUM
bank, so the packed O accumulators carry exactly one start per bank.
"""

import math

import numpy as np

import concourse.mybir as mybir
import concourse.tile as tile
from concourse import bacc
from concourse.masks import make_identity

F32 = mybir.dt.float32
F16 = mybir.dt.float16
AF = mybir.ActivationFunctionType

B, S, D, H = 4, 4096, 1024, 128
SQ = S // 2  # queries per core
N_CORES = 8

TWO_PI = 2.0 * math.pi
INV_TWO_PI = 1.0 / TWO_PI
MAGIC = 12582912.0  # 1.5 * 2**23: fp32 (u + M) - M == round(u)
INV_SQRT_H = 1.0 / math.sqrt(H)


def _cody_waite_consts():
    # Split 2*pi into c1 + c2 + c3, c1/c2 with zeroed low mantissa bits so
    # theta - k*c1 - k*c2 - k*c3 cancels exactly for small integer k.
    def chop(v):
        f = np.float32(v)
        i = f.view(np.uint32) & np.uint32(0xFFFFF000)
        return float(i.view(np.float32))

    c1 = chop(TWO_PI)
    c2 = chop(TWO_PI - c1)
    c3 = float(np.float32(TWO_PI - c1 - c2))
    return c1, c2, c3


C1, C2, C3 = _cody_waite_consts()

_CACHED = None


def _build():
    nc = bacc.Bacc("TRN2", target_bir_lowering=False, debug=False,
                   num_devices=N_CORES)

    xT = nc.dram_tensor("xT", (D, S), F16, kind="ExternalInput")
    WqTd = nc.dram_tensor("WqT", (D, H), F16, kind="ExternalInput")
    WkTd = nc.dram_tensor("WkT", (D, H), F16, kind="ExternalInput")
    WvTd = nc.dram_tensor("WvT", (D, H), F16, kind="ExternalInput")
    WoTd = nc.dram_tensor("WoT", (H, D), F16, kind="ExternalInput")
    vecs = nc.dram_tensor("vecs", (H, 5), F32, kind="ExternalInput")
    bo = nc.dram_tensor("bo", (1, D), F32, kind="ExternalInput")
    y = nc.dram_tensor("y", (SQ, D), F32, kind="ExternalOutput")

    with tile.TileContext(nc) as tc, \
            tc.tile_pool(name="const", bufs=1) as const, \
            tc.tile_pool(name="big", bufs=1) as big, \
            tc.tile_pool(name="xa", bufs=2) as xa_pool, \
            tc.tile_pool(name="tmp", bufs=3) as tmp:

        # ---- x.T chunk loads (plain DMA; host pre-transposed) ---------
        xT3 = xT.ap().rearrange("(o p) s -> p o s", p=128)
        xt_tiles = []
        for sc in range(8):
            xt = xa_pool.tile([128, 8, 512], F16, tag="xt", bufs=4,
                              name=f"xt_{sc}")
            nc.sync.dma_start(xt, xT3[:, :, sc * 512:(sc + 1) * 512])
            xt_tiles.append(xt)

        psum_t = tc.alloc_tile_pool(name="psum_a", bufs=2, space="PSUM")

        # HAM warm-up: the PE sits idle ~8us at startup waiting for the
        # first weight/x.T DMAs; fill that window with discarded matmuls
        # on a zeroed scratch tile so the clock gate opens (needs ~3.4us
        # of sustained activity) before the real projections begin.
        warm = const.tile([128, 512], F16)
        nc.vector.memset(warm, 0.0)
        wpp = psum_t.tile([128, 512], F32, tag="proj", bufs=6, name="warm_pp")
        for i in range(12):
            nc.tensor.matmul(wpp, warm[:, 0:128], warm,
                             start=(i == 0), stop=(i == 11),
                             skip_group_check=True)

        WkT = const.tile([128, 8, 128], F16)
        nc.scalar.dma_start(WkT, WkTd.ap().rearrange("(o p) h -> p o h", p=128))
        WvT = const.tile([128, 8, 128], F16)
        nc.scalar.dma_start(WvT, WvTd.ap().rearrange("(o p) h -> p o h", p=128))
        WqT = const.tile([128, 8, 128], F16)
        nc.scalar.dma_start(WqT, WqTd.ap().rearrange("(o p) h -> p o h", p=128))
        WoT = const.tile([128, D], F16)  # [h, d]
        nc.scalar.dma_start(WoT, WoTd.ap())

        # ---- constants -------------------------------------------------
        ident_h = const.tile([128, 128], F16)
        make_identity(nc, ident_h)

        vecs_sb = const.tile([H, 5], F32)
        nc.gpsimd.dma_start(vecs_sb, vecs.ap())
        wav_sb = vecs_sb[:, 0:1]
        phase_sb = vecs_sb[:, 1:2]
        bq_sb = vecs_sb[:, 2:3]
        bk_sb = vecs_sb[:, 3:4]
        bv_sb = vecs_sb[:, 4:5]

        inv_w = const.tile([H, 1], F32)
        tw = const.tile([H, 1], F32)
        nc.vector.tensor_scalar(tw, wav_sb, 1e-8, None, mybir.AluOpType.add)
        nc.vector.reciprocal(inv_w, tw)
        cadd_q = const.tile([H, 1], F32)
        nc.vector.tensor_scalar(cadd_q, bq_sb, inv_w, phase_sb,
                                mybir.AluOpType.mult, mybir.AluOpType.add)
        cadd_k = const.tile([H, 1], F32)
        nc.vector.tensor_scalar(cadd_k, bk_sb, inv_w, phase_sb,
                                mybir.AluOpType.mult, mybir.AluOpType.add)
        inv_w2 = const.tile([H, 1], F32)
        nc.vector.tensor_scalar(inv_w2, inv_w, INV_TWO_PI, None,
                                mybir.AluOpType.mult)
        cadd_q2 = const.tile([H, 1], F32)
        nc.vector.tensor_scalar(cadd_q2, cadd_q, INV_TWO_PI, None,
                                mybir.AluOpType.mult)
        cadd_k2 = const.tile([H, 1], F32)
        nc.vector.tensor_scalar(cadd_k2, cadd_k, INV_TWO_PI, None,
                                mybir.AluOpType.mult)

        neg1 = const.tile([128, 1], F32)
        nc.vector.memset(neg1, -1.0)
        cadd_q2M = const.tile([H, 1], F32)
        nc.vector.tensor_scalar(cadd_q2M, cadd_q2, MAGIC, None,
                                mybir.AluOpType.add)
        cadd_k2M = const.tile([H, 1], F32)
        nc.vector.tensor_scalar(cadd_k2M, cadd_k2, MAGIC, None,
                                mybir.AluOpType.add)
        negM = const.tile([128, 1], F32)
        nc.vector.memset(negM, -MAGIC)

        bo_row = const.tile([1, D], F32)
        nc.gpsimd.dma_start(bo_row, bo.ap())
        bo_tile = const.tile([128, D], F32)
        nc.gpsimd.partition_broadcast(bo_tile, bo_row)

        # ---- persistent activations -----------------------------------
        Fq_cos = big.tile([128, SQ], F16)
        Fq_sin = big.tile([128, SQ], F16)
        Fk_cos = big.tile([128, S], F16)
        Fk_sin = big.tile([128, S], F16)
        Vn = big.tile([128, 32, 129], F16)  # [k_part, k_tile, h | ones]
        nc.vector.memset(Vn[:, :, 128:129], 1.0)
        osb = big.tile([128, 16, 129], F32)  # raw [O | denom] per q-subtile
        recs = [big.tile([128, 1], F32, name=f"rec_{i}", tag=f"rec_{i}")
                for i in range(16)]

        # ---- phase A: x.T (DMA), projections, sin/cos, V --------------
        def theta_path(pp, cadd, cadd2M_, cos_slice, sin_slice):
            th = tmp.tile([128, 512], F32, tag="th")
            nc.vector.tensor_scalar(th, pp, inv_w, cadd,
                                    mybir.AluOpType.mult, mybir.AluOpType.add)
            u = tmp.tile([128, 512], F32, tag="u")
            nc.scalar.activation(u, pp, AF.Identity, bias=cadd2M_,
                                 scale=inv_w2)
            kk = tmp.tile([128, 512], F32, tag="kk")
            nc.scalar.activation(kk, u, AF.Identity, bias=negM,
                                 scale=1.0)
            thr = tmp.tile([128, 512], F32, tag="thr")
            nc.vector.cody_waite_cascade(thr, th, kk, C1, C2, C3)
            nc.scalar.activation(sin_slice, thr, AF.Sin)
            thc = tmp.tile([128, 512], F32, tag="thc")
            nc.vector.add_range_wrap(thc, thr, math.pi / 2, math.pi, TWO_PI)
            nc.scalar.activation(cos_slice, thc, AF.Sin)

        for sc in range(8):
            xt = xt_tiles[sc]

            def proj(wt):
                pp = psum_t.tile([128, 512], F32, tag="proj", bufs=6)
                for dc in range(8):
                    nc.tensor.matmul(pp, wt[:, dc, :], xt[:, dc, :],
                                     start=(dc == 0), stop=(dc == 7))
                return pp

            sl = slice(sc * 512, (sc + 1) * 512)
            theta_path(proj(WkT), cadd_k, cadd_k2M,
                       Fk_cos[:, sl], Fk_sin[:, sl])

            ppv = proj(WvT)
            v16 = tmp.tile([128, 512], F16, tag="v16")
            nc.scalar.activation(v16, ppv, AF.Identity, bias=bv_sb)

            if sc < 4:
                theta_path(proj(WqT), cadd_q, cadd_q2M,
                           Fq_cos[:, sl], Fq_sin[:, sl])

            pv = psum_t.tile([128, 512], F16, tag="pt")
            for a in range(4):
                nc.tensor.transpose(pv[:, a * 128:(a + 1) * 128],
                                    v16[:, a * 128:(a + 1) * 128], ident_h)
            nc.vector.tensor_copy(
                Vn[:, sc * 4:(sc + 1) * 4, 0:128],
                pv.rearrange("p (a h) -> p a h", a=4))

        psum_t.release()

        # ---- phase B: attention per 512-query chunk -------------------
        psum_b = tc.alloc_tile_pool(name="psum_b", bufs=1, space="PSUM")
        for qc in range(4):
            qsl = slice(qc * 512, (qc + 1) * 512)
            opsA = psum_b.tile([128, 3, 132], F32, tag="opsA",
                               name=f"opsA_{qc}")
            opsB = psum_b.tile([128, 129], F32, tag="opsB",
                               name=f"opsB_{qc}")
            ops = [opsA[:, 0, 0:129], opsA[:, 1, 0:129], opsA[:, 2, 0:129],
                   opsB]
            for kt2 in range(16):
                st = psum_b.tile([128, 1024], F32, tag="mm1k", bufs=3)
                for j in range(2):
                    kt = kt2 * 2 + j
                    ksl = slice(kt * 128, (kt + 1) * 128)
                    ssl = slice(j * 512, (j + 1) * 512)
                    nc.tensor.matmul(st[:, ssl], Fk_cos[:, ksl],
                                     Fq_cos[:, qsl], start=True, stop=False)
                    nc.tensor.matmul(st[:, ssl], Fk_sin[:, ksl],
                                     Fq_sin[:, qsl], start=False, stop=True)
                et = tmp.tile([128, 1024], F16, tag="et", bufs=3)
                nc.scalar.activation(et, st, AF.Exp, bias=neg1,
                                     scale=INV_SQRT_H)
                for j in range(2):
                    kt = kt2 * 2 + j
                    for qs in range(4):
                        # start=True zeroes the whole 2KB PSUM bank, so only
                        # the first write into opsA's bank may carry it.
                        nc.tensor.matmul(
                            ops[qs],
                            et[:, j * 512 + qs * 128:j * 512 + (qs + 1) * 128],
                            Vn[:, kt, :],
                            start=(kt == 0 and (qs == 0 or qs == 3)),
                            stop=(kt == 31),
                            skip_group_check=True)
            for qs in range(4):
                i = qc * 4 + qs
                nc.vector.tensor_copy(osb[:, i, :], ops[qs])
                nc.vector.reciprocal(recs[i], osb[:, i, 128:129])

        psum_b.release()

        # ---- phase C: normalize + output projection -------------------
        psum_c = tc.alloc_tile_pool(name="psum_c", bufs=1, space="PSUM")
        bo16 = const.tile([128, D], F16)
        nc.vector.tensor_copy(bo16, bo_tile)
        for qc in range(4):
            for qs in range(4):
                i = qc * 4 + qs
                onrm = tmp.tile([128, 128], F16, tag="onrm", bufs=4)
                nc.scalar.activation(onrm, osb[:, i, 0:128], AF.Copy,
                                     scale=recs[i])
                otp = psum_c.tile([128, 128], F16, tag="ptc", bufs=4)
                nc.tensor.transpose(otp, onrm, ident_h)
                ot = tmp.tile([128, 128], F16, tag="ot", bufs=4)
                nc.vector.tensor_copy(ot, otp)
                row = i * 128
                for half in range(2):
                    yp = psum_c.tile([128, 512], F32, tag="yp", bufs=4)
                    nc.tensor.matmul(yp, ident_h,
                                     bo16[:, half * 512:(half + 1) * 512],
                                     start=True, stop=False,
                                     skip_group_check=True)
                    nc.tensor.matmul(yp, ot,
                                     WoT[:, half * 512:(half + 1) * 512],
                                     start=False, stop=True,
                                     skip_group_check=True)
                    ysb = tmp.tile([128, 512], F32, tag="ysb", bufs=4)
                    if (qs + half) % 2 == 0:
                        nc.vector.tensor_copy(ysb, yp)
                    else:
                        nc.scalar.copy(ysb, yp)
                    eng = nc.sync if half == 0 else nc.scalar
                    eng.dma_start(
                        y.ap()[row:row + 128,
                               half * 512:(half + 1) * 512], ysb)
        psum_c.release()

    nc.compile()
    return nc


def get_nc():
    global _CACHED
    if _CACHED is None:
        _CACHED = _build()
    return _CACHED


def _in_maps(inputs):
    x = np.asarray(inputs["x"], np.float32)
    small = {
        "WqT": np.ascontiguousarray(np.asarray(inputs["Wq"], np.float16).T),
        "WkT": np.ascontiguousarray(np.asarray(inputs["Wk"], np.float16).T),
        "WvT": np.ascontiguousarray(np.asarray(inputs["Wv"], np.float16).T),
        "WoT": np.ascontiguousarray(np.asarray(inputs["Wo"], np.float16).T),
        "vecs": np.stack([
            np.asarray(inputs["wavelengths"], np.float32),
            np.asarray(inputs["phase_bias"], np.float32),
            np.asarray(inputs["bq"], np.float32),
            np.asarray(inputs["bk"], np.float32),
            np.asarray(inputs["bv"], np.float32),
        ], axis=1),
        "bo": np.asarray(inputs["bo"], np.float32).reshape(1, D),
    }
    maps = []
    for c in range(N_CORES):
        b, qoff = c // 2, (c % 2) * SQ
        xc = np.roll(x[b], -qoff, axis=0) if qoff else x[b]
        maps.append({"xT": np.ascontiguousarray(xc.astype(np.float16).T),
                     **small})
    return maps


def kernel(**inputs):
    from concourse.bass_utils import run_bass_kernel_spmd

    nc = get_nc()
    res = run_bass_kernel_spmd(nc, _in_maps(inputs),
                               core_ids=list(range(N_CORES)))
    out = np.empty((B, S, D), np.float32)
    for c in range(N_CORES):
        b, qoff = c // 2, (c % 2) * SQ
        out[b, qoff:qoff + SQ] = res.results[c]["y"]
    return out


# revision 55
# speedup vs baseline: 1.0154x; 1.0154x over previous
"""EulerAttentionHead Trainium2 kernel (8 NeuronCores, SPMD).

Reference computation (B=4, S=4096, D=1024, H=128):
    Q = x @ Wq.T + bq ; K = x @ Wk.T + bk ; V = x @ Wv.T + bv
    theta_{q,k} = {Q,K} / (wavelengths + 1e-8) + phase_bias
    sim = cos(tq) @ cos(tk).T + sin(tq) @ sin(tk).T
    out = softmax(sim / sqrt(H)) @ V @ Wo.T + bo

Sharding: 8 cores = 4 batches x 2 query-halves. Each core handles one
batch's full key/value set (4096 keys) and 2048 queries. The host rolls
x so each core's query rows are rows 0:2048 of its input (softmax over
keys is permutation-invariant, so key order doesn't matter).

Host prep: x and the four weight matrices are cast to fp16 and
pre-transposed in numpy, so every device-side matmul operand already has
its contraction dim on SBUF partitions and all device DMAs are plain
contiguous loads (no xbar transposes).

Per-core dataflow (PE datapath fp16, fp32 PSUM accumulation):
  phase A: per 512-row chunk, Q.T/K.T/V.T = W.T-stationary matmuls over
    x.T; theta built with per-partition scale/bias; round(theta/2pi) via
    the fp32 magic-number trick; Cody-Waite cascade + add_range_wrap
    (custom DVE ops) reduce into the ACT Sin LUT domain [-pi, pi];
    cos(t) = sin(wrap(t + pi/2)). V is re-transposed to natural [k, h]
    layout on the PE with a ones column appended.
  phase B: per 512-query chunk, S.T tiles [k,128 x q,512] = Fk-stationary
    matmuls (two k-tiles paired per 2-bank PSUM tile so one ACT pass
    computes exp(S/sqrt(H) - 1) -> E.T fp16; the -1 keeps exp under fp16
    max and cancels in the softmax normalization). AV: lhsT = E.T (FWL),
    rhs = [V | ones], so the softmax denominator accumulates as PSUM
    column 128 for free. Raw [O | denom] is evicted to SBUF; the
    reciprocal runs on the otherwise-idle DVE during B.
  phase C: normalize O rows by recip (ACT scale during eviction),
    PE-transpose O, project with Wo.T after seeding PSUM with bo via an
    identity matmul, evict on alternating DVE/ACT, store on alternating
    HWDGE queues.

PSUM is rebalanced per phase with stack-scoped tile pools (A: proj+V
transpose, B: 3 double-bank S.T tiles + packed O accumulators, C: O
transpose + output tiles). Note start=True zeroes the whole 2KB PSUM
bank, so the packed O accumulators carry exactly one start per bank.
"""

import math

import numpy as np

import concourse.mybir as mybir
import concourse.tile as tile
from concourse import bacc
from concourse.masks import make_identity

F32 = mybir.dt.float32
F16 = mybir.dt.float16
AF = mybir.ActivationFunctionType

B, S, D, H = 4, 4096, 1024, 128
SQ = S // 2  # queries per core
N_CORES = 8

TWO_PI = 2.0 * math.pi
INV_TWO_PI = 1.0 / TWO_PI
MAGIC = 12582912.0  # 1.5 * 2**23: fp32 (u + M) - M == round(u)
INV_SQRT_H = 1.0 / math.sqrt(H)


def _cody_waite_consts():
    # Split 2*pi into c1 + c2 + c3, c1/c2 with zeroed low mantissa bits so
    # theta - k*c1 - k*c2 - k*c3 cancels exactly for small integer k.
    def chop(v):
        f = np.float32(v)
        i = f.view(np.uint32) & np.uint32(0xFFFFF000)
        return float(i.view(np.float32))

    c1 = chop(TWO_PI)
    c2 = chop(TWO_PI - c1)
    c3 = float(np.float32(TWO_PI - c1 - c2))
    return c1, c2, c3


C1, C2, C3 = _cody_waite_consts()

_CACHED = None


def _build():
    nc = bacc.Bacc("TRN2", target_bir_lowering=False, debug=False,
                   num_devices=N_CORES)

    xT = nc.dram_tensor("xT", (D, S), F16, kind="ExternalInput")
    WqTd = nc.dram_tensor("WqT", (D, H), F16, kind="ExternalInput")
    WkTd = nc.dram_tensor("WkT", (D, H), F16, kind="ExternalInput")
    WvTd = nc.dram_tensor("WvT", (D, H), F16, kind="ExternalInput")
    WoTd = nc.dram_tensor("WoT", (H, D), F16, kind="ExternalInput")
    vecs = nc.dram_tensor("vecs", (H, 5), F32, kind="ExternalInput")
    bo = nc.dram_tensor("bo", (1, D), F32, kind="ExternalInput")
    y = nc.dram_tensor("y", (SQ, D), F32, kind="ExternalOutput")

    with tile.TileContext(nc) as tc, \
            tc.tile_pool(name="const", bufs=1) as const, \
            tc.tile_pool(name="big", bufs=1) as big, \
            tc.tile_pool(name="xa", bufs=2) as xa_pool, \
            tc.tile_pool(name="tmp", bufs=3) as tmp:

        # ---- x.T chunk loads (plain DMA; host pre-transposed) ---------
        xT3 = xT.ap().rearrange("(o p) s -> p o s", p=128)
        xt_tiles = []
        for sc in range(8):
            xt = xa_pool.tile([128, 8, 512], F16, tag="xt", bufs=4,
                              name=f"xt_{sc}")
            nc.sync.dma_start(xt, xT3[:, :, sc * 512:(sc + 1) * 512])
            xt_tiles.append(xt)

        psum_t = tc.alloc_tile_pool(name="psum_a", bufs=2, space="PSUM")

        WkT = const.tile([128, 8, 128], F16)
        nc.scalar.dma_start(WkT, WkTd.ap().rearrange("(o p) h -> p o h", p=128))
        WvT = const.tile([128, 8, 128], F16)
        nc.scalar.dma_start(WvT, WvTd.ap().rearrange("(o p) h -> p o h", p=128))
        WqT = const.tile([128, 8, 128], F16)
        nc.scalar.dma_start(WqT, WqTd.ap().rearrange("(o p) h -> p o h", p=128))
        WoT = const.tile([128, D], F16)  # [h, d]
        nc.scalar.dma_start(WoT, WoTd.ap())

        # ---- constants -------------------------------------------------
        ident_h = const.tile([128, 128], F16)
        make_identity(nc, ident_h)

        vecs_sb = const.tile([H, 5], F32)
        nc.gpsimd.dma_start(vecs_sb, vecs.ap())
        wav_sb = vecs_sb[:, 0:1]
        phase_sb = vecs_sb[:, 1:2]
        bq_sb = vecs_sb[:, 2:3]
        bk_sb = vecs_sb[:, 3:4]
        bv_sb = vecs_sb[:, 4:5]

        inv_w = const.tile([H, 1], F32)
        tw = const.tile([H, 1], F32)
        nc.vector.tensor_scalar(tw, wav_sb, 1e-8, None, mybir.AluOpType.add)
        nc.vector.reciprocal(inv_w, tw)
        cadd_q = const.tile([H, 1], F32)
        nc.vector.tensor_scalar(cadd_q, bq_sb, inv_w, phase_sb,
                                mybir.AluOpType.mult, mybir.AluOpType.add)
        cadd_k = const.tile([H, 1], F32)
        nc.vector.tensor_scalar(cadd_k, bk_sb, inv_w, phase_sb,
                                mybir.AluOpType.mult, mybir.AluOpType.add)
        inv_w2 = const.tile([H, 1], F32)
        nc.vector.tensor_scalar(inv_w2, inv_w, INV_TWO_PI, None,
                                mybir.AluOpType.mult)
        cadd_q2 = const.tile([H, 1], F32)
        nc.vector.tensor_scalar(cadd_q2, cadd_q, INV_TWO_PI, None,
                                mybir.AluOpType.mult)
        cadd_k2 = const.tile([H, 1], F32)
        nc.vector.tensor_scalar(cadd_k2, cadd_k, INV_TWO_PI, None,
                                mybir.AluOpType.mult)

        neg1 = const.tile([128, 1], F32)
        nc.vector.memset(neg1, -1.0)
        cadd_q2M = const.tile([H, 1], F32)
        nc.vector.tensor_scalar(cadd_q2M, cadd_q2, MAGIC, None,
                                mybir.AluOpType.add)
        cadd_k2M = const.tile([H, 1], F32)
        nc.vector.tensor_scalar(cadd_k2M, cadd_k2, MAGIC, None,
                                mybir.AluOpType.add)
        negM = const.tile([128, 1], F32)
        nc.vector.memset(negM, -MAGIC)

        bo_row = const.tile([1, D], F32)
        nc.gpsimd.dma_start(bo_row, bo.ap())
        bo_tile = const.tile([128, D], F32)
        nc.gpsimd.partition_broadcast(bo_tile, bo_row)

        # ---- persistent activations -----------------------------------
        Fq_cos = big.tile([128, SQ], F16)
        Fq_sin = big.tile([128, SQ], F16)
        Fk_cos = big.tile([128, S], F16)
        Fk_sin = big.tile([128, S], F16)
        Vn = big.tile([128, 32, 129], F16)  # [k_part, k_tile, h | ones]
        nc.vector.memset(Vn[:, :, 128:129], 1.0)
        osb = big.tile([128, 16, 129], F32)  # raw [O | denom] per q-subtile
        recs = [big.tile([128, 1], F32, name=f"rec_{i}", tag=f"rec_{i}")
                for i in range(16)]

        # ---- phase A: x.T (DMA), projections, sin/cos, V --------------
        def theta_path(pp, cadd, cadd2M_, cos_slice, sin_slice):
            th = tmp.tile([128, 512], F32, tag="th", bufs=4)
            nc.vector.tensor_scalar(th, pp, inv_w, cadd,
                                    mybir.AluOpType.mult, mybir.AluOpType.add)
            u = tmp.tile([128, 512], F32, tag="u", bufs=4)
            nc.scalar.activation(u, pp, AF.Identity, bias=cadd2M_,
                                 scale=inv_w2)
            kk = tmp.tile([128, 512], F32, tag="kk", bufs=4)
            nc.scalar.activation(kk, u, AF.Identity, bias=negM,
                                 scale=1.0)
            thr = tmp.tile([128, 512], F32, tag="thr", bufs=4)
            nc.vector.cody_waite_cascade(thr, th, kk, C1, C2, C3)
            nc.scalar.activation(sin_slice, thr, AF.Sin)
            thc = tmp.tile([128, 512], F32, tag="thc", bufs=4)
            nc.vector.add_range_wrap(thc, thr, math.pi / 2, math.pi, TWO_PI)
            nc.scalar.activation(cos_slice, thc, AF.Sin)

        for sc in range(8):
            xt = xt_tiles[sc]

            def proj(wt):
                pp = psum_t.tile([128, 512], F32, tag="proj", bufs=6)
                for dc in range(8):
                    nc.tensor.matmul(pp, wt[:, dc, :], xt[:, dc, :],
                                     start=(dc == 0), stop=(dc == 7))
                return pp

            sl = slice(sc * 512, (sc + 1) * 512)
            theta_path(proj(WkT), cadd_k, cadd_k2M,
                       Fk_cos[:, sl], Fk_sin[:, sl])

            ppv = proj(WvT)
            v16 = tmp.tile([128, 512], F16, tag="v16")
            nc.scalar.activation(v16, ppv, AF.Identity, bias=bv_sb)

            if sc < 4:
                theta_path(proj(WqT), cadd_q, cadd_q2M,
                           Fq_cos[:, sl], Fq_sin[:, sl])

            pv = psum_t.tile([128, 512], F16, tag="pt")
            for a in range(4):
                nc.tensor.transpose(pv[:, a * 128:(a + 1) * 128],
                                    v16[:, a * 128:(a + 1) * 128], ident_h)
            nc.vector.tensor_copy(
                Vn[:, sc * 4:(sc + 1) * 4, 0:128],
                pv.rearrange("p (a h) -> p a h", a=4))

        psum_t.release()

        # ---- phase B: attention per 512-query chunk -------------------
        psum_b = tc.alloc_tile_pool(name="psum_b", bufs=1, space="PSUM")
        for qc in range(4):
            qsl = slice(qc * 512, (qc + 1) * 512)
            opsA = psum_b.tile([128, 3, 132], F32, tag="opsA",
                               name=f"opsA_{qc}")
            opsB = psum_b.tile([128, 129], F32, tag="opsB",
                               name=f"opsB_{qc}")
            ops = [opsA[:, 0, 0:129], opsA[:, 1, 0:129], opsA[:, 2, 0:129],
                   opsB]
            for kt2 in range(16):
                st = psum_b.tile([128, 1024], F32, tag="mm1k", bufs=3)
                for j in range(2):
                    kt = kt2 * 2 + j
                    ksl = slice(kt * 128, (kt + 1) * 128)
                    ssl = slice(j * 512, (j + 1) * 512)
                    nc.tensor.matmul(st[:, ssl], Fk_cos[:, ksl],
                                     Fq_cos[:, qsl], start=True, stop=False)
                    nc.tensor.matmul(st[:, ssl], Fk_sin[:, ksl],
                                     Fq_sin[:, qsl], start=False, stop=True)
                et = tmp.tile([128, 1024], F16, tag="et", bufs=3)
                nc.scalar.activation(et, st, AF.Exp, bias=neg1,
                                     scale=INV_SQRT_H)
                for j in range(2):
                    kt = kt2 * 2 + j
                    for qs in range(4):
                        # start=True zeroes the whole 2KB PSUM bank, so only
                        # the first write into opsA's bank may carry it.
                        nc.tensor.matmul(
                            ops[qs],
                            et[:, j * 512 + qs * 128:j * 512 + (qs + 1) * 128],
                            Vn[:, kt, :],
                            start=(kt == 0 and (qs == 0 or qs == 3)),
                            stop=(kt == 31),
                            skip_group_check=True)
            for qs in range(4):
                i = qc * 4 + qs
                nc.vector.tensor_copy(osb[:, i, :], ops[qs])
                nc.vector.reciprocal(recs[i], osb[:, i, 128:129])

        psum_b.release()

        # ---- phase C: normalize + output projection -------------------
        psum_c = tc.alloc_tile_pool(name="psum_c", bufs=1, space="PSUM")
        bo16 = const.tile([128, D], F16)
        nc.vector.tensor_copy(bo16, bo_tile)
        for qc in range(4):
            for qs in range(4):
                i = qc * 4 + qs
                onrm = tmp.tile([128, 128], F16, tag="onrm", bufs=4)
                nc.scalar.activation(onrm, osb[:, i, 0:128], AF.Copy,
                                     scale=recs[i])
                otp = psum_c.tile([128, 128], F16, tag="ptc", bufs=4)
                nc.tensor.transpose(otp, onrm, ident_h)
                ot = tmp.tile([128, 128], F16, tag="ot", bufs=4)
                nc.vector.tensor_copy(ot, otp)
                row = i * 128
                for half in range(2):
                    yp = psum_c.tile([128, 512], F32, tag="yp", bufs=4)
                    dve_evict = (qs + half) % 2 == 0
                    if not dve_evict:
                        nc.tensor.matmul(yp, ident_h,
                                         bo16[:, half * 512:(half + 1) * 512],
                                         start=True, stop=False,
                                         skip_group_check=True)
                    nc.tensor.matmul(yp, ot,
                                     WoT[:, half * 512:(half + 1) * 512],
                                     start=dve_evict, stop=True,
                                     skip_group_check=True)
                    ysb = tmp.tile([128, 512], F32, tag="ysb", bufs=4)
                    if dve_evict:
                        nc.vector.tensor_add(
                            ysb, yp, bo_tile[:, half * 512:(half + 1) * 512])
                    else:
                        nc.scalar.copy(ysb, yp)
                    eng = nc.sync if half == 0 else nc.scalar
                    eng.dma_start(
                        y.ap()[row:row + 128,
                               half * 512:(half + 1) * 512], ysb)
        psum_c.release()

    nc.compile()
    return nc


def get_nc():
    global _CACHED
    if _CACHED is None:
        _CACHED = _build()
    return _CACHED


def _in_maps(inputs):
    x = np.asarray(inputs["x"], np.float32)
    small = {
        "WqT": np.ascontiguousarray(np.asarray(inputs["Wq"], np.float16).T),
        "WkT": np.ascontiguousarray(np.asarray(inputs["Wk"], np.float16).T),
        "WvT": np.ascontiguousarray(np.asarray(inputs["Wv"], np.float16).T),
        "WoT": np.ascontiguousarray(np.asarray(inputs["Wo"], np.float16).T),
        "vecs": np.stack([
            np.asarray(inputs["wavelengths"], np.float32),
            np.asarray(inputs["phase_bias"], np.float32),
            np.asarray(inputs["bq"], np.float32),
            np.asarray(inputs["bk"], np.float32),
            np.asarray(inputs["bv"], np.float32),
        ], axis=1),
        "bo": np.asarray(inputs["bo"], np.float32).reshape(1, D),
    }
    maps = []
    for c in range(N_CORES):
        b, qoff = c // 2, (c % 2) * SQ
        xc = np.roll(x[b], -qoff, axis=0) if qoff else x[b]
        maps.append({"xT": np.ascontiguousarray(xc.astype(np.float16).T),
                     **small})
    return maps


def kernel(**inputs):
    from concourse.bass_utils import run_bass_kernel_spmd

    nc = get_nc()
    res = run_bass_kernel_spmd(nc, _in_maps(inputs),
                               core_ids=list(range(N_CORES)))
    out = np.empty((B, S, D), np.float32)
    for c in range(N_CORES):
        b, qoff = c // 2, (c % 2) * SQ
        out[b, qoff:qoff + SQ] = res.results[c]["y"]
    return out


# revision 56
# speedup vs baseline: 1.0167x; 1.0013x over previous
"""EulerAttentionHead Trainium2 kernel (8 NeuronCores, SPMD).

Reference computation (B=4, S=4096, D=1024, H=128):
    Q = x @ Wq.T + bq ; K = x @ Wk.T + bk ; V = x @ Wv.T + bv
    theta_{q,k} = {Q,K} / (wavelengths + 1e-8) + phase_bias
    sim = cos(tq) @ cos(tk).T + sin(tq) @ sin(tk).T
    out = softmax(sim / sqrt(H)) @ V @ Wo.T + bo

Sharding: 8 cores = 4 batches x 2 query-halves. Each core handles one
batch's full key/value set (4096 keys) and 2048 queries. The host rolls
x so each core's query rows are rows 0:2048 of its input (softmax over
keys is permutation-invariant, so key order doesn't matter).

Host prep: x and the four weight matrices are cast to fp16 and
pre-transposed in numpy, so every device-side matmul operand already has
its contraction dim on SBUF partitions and all device DMAs are plain
contiguous loads (no xbar transposes).

Per-core dataflow (PE datapath fp16, fp32 PSUM accumulation):
  phase A: per 512-row chunk, Q.T/K.T/V.T = W.T-stationary matmuls over
    x.T; theta built with per-partition scale/bias; round(theta/2pi) via
    the fp32 magic-number trick; Cody-Waite cascade + add_range_wrap
    (custom DVE ops) reduce into the ACT Sin LUT domain [-pi, pi];
    cos(t) = sin(wrap(t + pi/2)). V is re-transposed to natural [k, h]
    layout on the PE with a ones column appended.
  phase B: per 512-query chunk, S.T tiles [k,128 x q,512] = Fk-stationary
    matmuls (two k-tiles paired per 2-bank PSUM tile so one ACT pass
    computes exp(S/sqrt(H) - 1) -> E.T fp16; the -1 keeps exp under fp16
    max and cancels in the softmax normalization). AV: lhsT = E.T (FWL),
    rhs = [V | ones], so the softmax denominator accumulates as PSUM
    column 128 for free. Raw [O | denom] is evicted to SBUF; the
    reciprocal runs on the otherwise-idle DVE during B.
  phase C: normalize O rows by recip (ACT scale during eviction),
    PE-transpose O, project with Wo.T after seeding PSUM with bo via an
    identity matmul, evict on alternating DVE/ACT, store on alternating
    HWDGE queues.

PSUM is rebalanced per phase with stack-scoped tile pools (A: proj+V
transpose, B: 3 double-bank S.T tiles + packed O accumulators, C: O
transpose + output tiles). Note start=True zeroes the whole 2KB PSUM
bank, so the packed O accumulators carry exactly one start per bank.
"""

import math

import numpy as np

import concourse.mybir as mybir
import concourse.tile as tile
from concourse import bacc
from concourse.masks import make_identity

F32 = mybir.dt.float32
F16 = mybir.dt.float16
AF = mybir.ActivationFunctionType

B, S, D, H = 4, 4096, 1024, 128
SQ = S // 2  # queries per core
N_CORES = 8

TWO_PI = 2.0 * math.pi
INV_TWO_PI = 1.0 / TWO_PI
MAGIC = 12582912.0  # 1.5 * 2**23: fp32 (u + M) - M == round(u)
INV_SQRT_H = 1.0 / math.sqrt(H)


def _cody_waite_consts():
    # Split 2*pi into c1 + c2 + c3, c1/c2 with zeroed low mantissa bits so
    # theta - k*c1 - k*c2 - k*c3 cancels exactly for small integer k.
    def chop(v):
        f = np.float32(v)
        i = f.view(np.uint32) & np.uint32(0xFFFFF000)
        return float(i.view(np.float32))

    c1 = chop(TWO_PI)
    c2 = chop(TWO_PI - c1)
    c3 = float(np.float32(TWO_PI - c1 - c2))
    return c1, c2, c3


C1, C2, C3 = _cody_waite_consts()

_CACHED = None


def _build():
    nc = bacc.Bacc("TRN2", target_bir_lowering=False, debug=False,
                   num_devices=N_CORES)

    xT = nc.dram_tensor("xT", (D, S), F16, kind="ExternalInput")
    WqTd = nc.dram_tensor("WqT", (D, H), F16, kind="ExternalInput")
    WkTd = nc.dram_tensor("WkT", (D, H), F16, kind="ExternalInput")
    WvTd = nc.dram_tensor("WvT", (D, H), F16, kind="ExternalInput")
    WoTd = nc.dram_tensor("WoT", (H, D), F16, kind="ExternalInput")
    vecs = nc.dram_tensor("vecs", (H, 5), F32, kind="ExternalInput")
    bo = nc.dram_tensor("bo", (1, D), F32, kind="ExternalInput")
    y = nc.dram_tensor("y", (SQ, D), F32, kind="ExternalOutput")

    with tile.TileContext(nc) as tc, \
            tc.tile_pool(name="const", bufs=1) as const, \
            tc.tile_pool(name="big", bufs=1) as big, \
            tc.tile_pool(name="xa", bufs=2) as xa_pool, \
            tc.tile_pool(name="tmp", bufs=3) as tmp:

        # ---- x.T chunk loads (plain DMA; host pre-transposed) ---------
        xT3 = xT.ap().rearrange("(o p) s -> p o s", p=128)
        xt_tiles = []
        for sc in range(8):
            xt = xa_pool.tile([128, 8, 512], F16, tag="xt", bufs=4,
                              name=f"xt_{sc}")
            nc.sync.dma_start(xt, xT3[:, :, sc * 512:(sc + 1) * 512])
            xt_tiles.append(xt)

        psum_t = tc.alloc_tile_pool(name="psum_a", bufs=2, space="PSUM")

        WkT = const.tile([128, 8, 128], F16)
        nc.scalar.dma_start(WkT, WkTd.ap().rearrange("(o p) h -> p o h", p=128))
        WvT = const.tile([128, 8, 128], F16)
        nc.scalar.dma_start(WvT, WvTd.ap().rearrange("(o p) h -> p o h", p=128))
        WqT = const.tile([128, 8, 128], F16)
        nc.scalar.dma_start(WqT, WqTd.ap().rearrange("(o p) h -> p o h", p=128))
        WoT = const.tile([128, D], F16)  # [h, d]
        nc.scalar.dma_start(WoT, WoTd.ap())

        # ---- constants -------------------------------------------------
        ident_h = const.tile([128, 128], F16)
        make_identity(nc, ident_h)

        vecs_sb = const.tile([H, 5], F32)
        nc.gpsimd.dma_start(vecs_sb, vecs.ap())
        wav_sb = vecs_sb[:, 0:1]
        phase_sb = vecs_sb[:, 1:2]
        bq_sb = vecs_sb[:, 2:3]
        bk_sb = vecs_sb[:, 3:4]
        bv_sb = vecs_sb[:, 4:5]

        inv_w = const.tile([H, 1], F32)
        tw = const.tile([H, 1], F32)
        nc.vector.tensor_scalar(tw, wav_sb, 1e-8, None, mybir.AluOpType.add)
        nc.vector.reciprocal(inv_w, tw)
        cadd_q = const.tile([H, 1], F32)
        nc.vector.tensor_scalar(cadd_q, bq_sb, inv_w, phase_sb,
                                mybir.AluOpType.mult, mybir.AluOpType.add)
        cadd_k = const.tile([H, 1], F32)
        nc.vector.tensor_scalar(cadd_k, bk_sb, inv_w, phase_sb,
                                mybir.AluOpType.mult, mybir.AluOpType.add)
        inv_w2 = const.tile([H, 1], F32)
        nc.vector.tensor_scalar(inv_w2, inv_w, INV_TWO_PI, None,
                                mybir.AluOpType.mult)
        cadd_q2 = const.tile([H, 1], F32)
        nc.vector.tensor_scalar(cadd_q2, cadd_q, INV_TWO_PI, None,
                                mybir.AluOpType.mult)
        cadd_k2 = const.tile([H, 1], F32)
        nc.vector.tensor_scalar(cadd_k2, cadd_k, INV_TWO_PI, None,
                                mybir.AluOpType.mult)

        neg1 = const.tile([128, 1], F32)
        nc.vector.memset(neg1, -1.0)
        cadd_q2M = const.tile([H, 1], F32)
        nc.vector.tensor_scalar(cadd_q2M, cadd_q2, MAGIC, None,
                                mybir.AluOpType.add)
        cadd_k2M = const.tile([H, 1], F32)
        nc.vector.tensor_scalar(cadd_k2M, cadd_k2, MAGIC, None,
                                mybir.AluOpType.add)
        negM = const.tile([128, 1], F32)
        nc.vector.memset(negM, -MAGIC)

        bo_row = const.tile([1, D], F32)
        nc.gpsimd.dma_start(bo_row, bo.ap())
        bo_tile = const.tile([128, D], F32)
        nc.gpsimd.partition_broadcast(bo_tile, bo_row)

        # ---- persistent activations -----------------------------------
        Fq_cos = big.tile([128, SQ], F16)
        Fq_sin = big.tile([128, SQ], F16)
        Fk_cos = big.tile([128, S], F16)
        Fk_sin = big.tile([128, S], F16)
        Vn = big.tile([128, 32, 129], F16)  # [k_part, k_tile, h | ones]
        nc.vector.memset(Vn[:, :, 128:129], 1.0)
        osb = big.tile([128, 16, 129], F32)  # raw [O | denom] per q-subtile
        recs = [big.tile([128, 1], F32, name=f"rec_{i}", tag=f"rec_{i}")
                for i in range(16)]

        # ---- phase A: x.T (DMA), projections, sin/cos, V --------------
        def theta_path(pp, cadd, cadd2M_, cos_slice, sin_slice):
            th = tmp.tile([128, 512], F32, tag="th", bufs=4)
            nc.vector.tensor_scalar(th, pp, inv_w, cadd,
                                    mybir.AluOpType.mult, mybir.AluOpType.add)
            u = tmp.tile([128, 512], F32, tag="u", bufs=4)
            nc.scalar.activation(u, pp, AF.Identity, bias=cadd2M_,
                                 scale=inv_w2)
            kk = tmp.tile([128, 512], F32, tag="kk", bufs=4)
            nc.scalar.activation(kk, u, AF.Identity, bias=negM,
                                 scale=1.0)
            thr = tmp.tile([128, 512], F32, tag="thr", bufs=4)
            nc.vector.cody_waite_cascade(thr, th, kk, C1, C2, C3)
            nc.scalar.activation(sin_slice, thr, AF.Sin)
            thc = tmp.tile([128, 512], F32, tag="thc", bufs=4)
            nc.vector.add_range_wrap(thc, thr, math.pi / 2, math.pi, TWO_PI)
            nc.scalar.activation(cos_slice, thc, AF.Sin)

        for sc in range(8):
            xt = xt_tiles[sc]

            def proj(wt):
                pp = psum_t.tile([128, 512], F32, tag="proj", bufs=6)
                for dc in range(8):
                    nc.tensor.matmul(pp, wt[:, dc, :], xt[:, dc, :],
                                     start=(dc == 0), stop=(dc == 7))
                return pp

            sl = slice(sc * 512, (sc + 1) * 512)
            theta_path(proj(WkT), cadd_k, cadd_k2M,
                       Fk_cos[:, sl], Fk_sin[:, sl])

            ppv = proj(WvT)
            v16 = tmp.tile([128, 512], F16, tag="v16")
            nc.scalar.activation(v16, ppv, AF.Identity, bias=bv_sb)

            if sc < 4:
                theta_path(proj(WqT), cadd_q, cadd_q2M,
                           Fq_cos[:, sl], Fq_sin[:, sl])

            pv = psum_t.tile([128, 512], F16, tag="pt")
            for a in range(4):
                nc.tensor.transpose(pv[:, a * 128:(a + 1) * 128],
                                    v16[:, a * 128:(a + 1) * 128], ident_h)
            nc.vector.tensor_copy(
                Vn[:, sc * 4:(sc + 1) * 4, 0:128],
                pv.rearrange("p (a h) -> p a h", a=4))

        psum_t.release()

        # ---- phase B: attention per 512-query chunk -------------------
        psum_b = tc.alloc_tile_pool(name="psum_b", bufs=1, space="PSUM")
        for qc in range(4):
            qsl = slice(qc * 512, (qc + 1) * 512)
            opsA = psum_b.tile([128, 3, 132], F32, tag="opsA",
                               name=f"opsA_{qc}")
            opsB = psum_b.tile([128, 129], F32, tag="opsB",
                               name=f"opsB_{qc}")
            ops = [opsA[:, 0, 0:129], opsA[:, 1, 0:129], opsA[:, 2, 0:129],
                   opsB]
            for kt2 in range(16):
                st = psum_b.tile([128, 1024], F32, tag="mm1k", bufs=3)
                for j in range(2):
                    kt = kt2 * 2 + j
                    ksl = slice(kt * 128, (kt + 1) * 128)
                    ssl = slice(j * 512, (j + 1) * 512)
                    nc.tensor.matmul(st[:, ssl], Fk_cos[:, ksl],
                                     Fq_cos[:, qsl], start=True, stop=False)
                    nc.tensor.matmul(st[:, ssl], Fk_sin[:, ksl],
                                     Fq_sin[:, qsl], start=False, stop=True)
                et = tmp.tile([128, 1024], F16, tag="et", bufs=3)
                nc.scalar.activation(et, st, AF.Exp, bias=neg1,
                                     scale=INV_SQRT_H)
                for j in range(2):
                    kt = kt2 * 2 + j
                    for qs in range(4):
                        # start=True zeroes the whole 2KB PSUM bank, so only
                        # the first write into opsA's bank may carry it.
                        nc.tensor.matmul(
                            ops[qs],
                            et[:, j * 512 + qs * 128:j * 512 + (qs + 1) * 128],
                            Vn[:, kt, :],
                            start=(kt == 0 and (qs == 0 or qs == 3)),
                            stop=(kt == 31),
                            skip_group_check=True)
            nc.vector.tensor_copy(osb[:, qc * 4:qc * 4 + 3, :],
                                  opsA[:, :, 0:129])
            nc.vector.tensor_copy(osb[:, qc * 4 + 3, :], opsB)
            for qs in range(4):
                i = qc * 4 + qs
                nc.vector.reciprocal(recs[i], osb[:, i, 128:129])

        psum_b.release()

        # ---- phase C: normalize + output projection -------------------
        psum_c = tc.alloc_tile_pool(name="psum_c", bufs=1, space="PSUM")
        bo16 = const.tile([128, D], F16)
        nc.vector.tensor_copy(bo16, bo_tile)
        for qc in range(4):
            for qs in range(4):
                i = qc * 4 + qs
                onrm = tmp.tile([128, 128], F16, tag="onrm", bufs=4)
                nc.scalar.activation(onrm, osb[:, i, 0:128], AF.Copy,
                                     scale=recs[i])
                otp = psum_c.tile([128, 128], F16, tag="ptc", bufs=4)
                nc.tensor.transpose(otp, onrm, ident_h)
                ot = tmp.tile([128, 128], F16, tag="ot", bufs=4)
                nc.vector.tensor_copy(ot, otp)
                row = i * 128
                for half in range(2):
                    yp = psum_c.tile([128, 512], F32, tag="yp", bufs=4)
                    dve_evict = (qs + half) % 2 == 0
                    if not dve_evict:
                        nc.tensor.matmul(yp, ident_h,
                                         bo16[:, half * 512:(half + 1) * 512],
                                         start=True, stop=False,
                                         skip_group_check=True)
                    nc.tensor.matmul(yp, ot,
                                     WoT[:, half * 512:(half + 1) * 512],
                                     start=dve_evict, stop=True,
                                     skip_group_check=True)
                    ysb = tmp.tile([128, 512], F32, tag="ysb", bufs=4)
                    if dve_evict:
                        nc.vector.tensor_add(
                            ysb, yp, bo_tile[:, half * 512:(half + 1) * 512])
                    else:
                        nc.scalar.copy(ysb, yp)
                    eng = nc.sync if half == 0 else nc.scalar
                    eng.dma_start(
                        y.ap()[row:row + 128,
                               half * 512:(half + 1) * 512], ysb)
        psum_c.release()

    nc.compile()
    return nc


def get_nc():
    global _CACHED
    if _CACHED is None:
        _CACHED = _build()
    return _CACHED


def _in_maps(inputs):
    x = np.asarray(inputs["x"], np.float32)
    small = {
        "WqT": np.ascontiguousarray(np.asarray(inputs["Wq"], np.float16).T),
        "WkT": np.ascontiguousarray(np.asarray(inputs["Wk"], np.float16).T),
        "WvT": np.ascontiguousarray(np.asarray(inputs["Wv"], np.float16).T),
        "WoT": np.ascontiguousarray(np.asarray(inputs["Wo"], np.float16).T),
        "vecs": np.stack([
            np.asarray(inputs["wavelengths"], np.float32),
            np.asarray(inputs["phase_bias"], np.float32),
            np.asarray(inputs["bq"], np.float32),
            np.asarray(inputs["bk"], np.float32),
            np.asarray(inputs["bv"], np.float32),
        ], axis=1),
        "bo": np.asarray(inputs["bo"], np.float32).reshape(1, D),
    }
    maps = []
    for c in range(N_CORES):
        b, qoff = c // 2, (c % 2) * SQ
        xc = np.roll(x[b], -qoff, axis=0) if qoff else x[b]
        maps.append({"xT": np.ascontiguousarray(xc.astype(np.float16).T),
                     **small})
    return maps


def kernel(**inputs):
    from concourse.bass_utils import run_bass_kernel_spmd

    nc = get_nc()
    res = run_bass_kernel_spmd(nc, _in_maps(inputs),
                               core_ids=list(range(N_CORES)))
    out = np.empty((B, S, D), np.float32)
    for c in range(N_CORES):
        b, qoff = c // 2, (c % 2) * SQ
        out[b, qoff:qoff + SQ] = res.results[c]["y"]
    return out
